# revision 1
# baseline (speedup 1.0000x reference)
"""BitNet-style attention layer (B=2, T=2048, D=1024, 16 heads, RoPE, causal)
on 8 TRN2 NeuronCores.

Sharding: head-parallel attention (2 heads/core); wo is computed per-core for
an o-slice after an AllGather of the int8-valued (bf16-stored) quantized
attention output.  One small AllReduce(max) provides the wo-input
quantization scale.
"""

import math
from contextlib import ExitStack

import ml_dtypes
import numpy as np

import concourse.bass as bass
import concourse.bacc as bacc_mod
import concourse.mybir as mybir
import concourse.tile as tile
from concourse.bass_utils import run_bass_kernel_spmd

F32 = mybir.dt.float32
F32R = mybir.dt.float32r
BF16 = mybir.dt.bfloat16
OP = mybir.AluOpType

B, T, D = 2, 2048, 1024
NT = B * T              # 4096 tokens
NH, HD = 16, 64
N_CORES = 8
HPC = NH // N_CORES     # heads per core = 2
DPC = HPC * HD          # dims per core = 128
RC = 12582912.0         # 1.5*2^23: round-to-nearest-even constant
NEG = -1e30

TB = 512                # token block (matmul N)
NTB = NT // TB          # 8
NTT = NT // 128         # 32 token tiles
QB = 512                # q block
NQB = T // QB           # 4 per batch
NKT = T // 128          # 16 k tiles per batch


def _quant_w(w):
    O, I = w.shape
    wg = w.reshape(O, I // 128, 128)
    ws = np.abs(wg).mean(-1, keepdims=True) + 1e-5
    wq = np.clip(np.round(wg / ws), -1.0, 1.0) * ws
    return wq.reshape(O, I).astype(np.float32)


def build_nc():
    nc = bacc_mod.Bacc(num_devices=N_CORES)
    io = {}

    def inp(name, shape, dt=F32):
        io[name] = nc.dram_tensor(name, shape, dt, kind="ExternalInput")

    inp("x", [NT, D])
    inp("sxp", [128, NTT])
    inp("isx", [128, NT])
    inp("wqT", [D, DPC], F32R)
    inp("wkT", [D, DPC], F32R)
    inp("wvT", [D, DPC], F32R)
    inp("woT", [D, DPC], BF16)
    inp("cmap", [128, NT])
    inp("smap", [128, NT])
    inp("pswapT", [128, 128], F32R)
    inp("negI", [128, 128], BF16)
    inp("umask", [128, 4 * QB], BF16)
    inp("sel2", [33, 128], F32R)
    inp("ones1", [1, 128], F32R)
    inp("ident", [128, 128], F32R)
    out = nc.dram_tensor("out", [DPC, NT], F32, kind="ExternalOutput")

    r32 = lambda ap: ap.bitcast(F32R)
    RG = [list(range(N_CORES))]

    with nc.allow_low_precision(reason="f32r matmul pipeline (FP22 mantissa is ample here)"), \
         tile.TileContext(nc) as tc, ExitStack() as top:
        cpool = top.enter_context(tc.tile_pool(name="const", bufs=1))
        dpool = top.enter_context(tc.tile_pool(name="dram", bufs=1, space="DRAM"))

        # ---- constants
        def const_tile(name, shape, dt=F32, src=None):
            t = cpool.tile(shape, dt, tag=name)
            nc.gpsimd.dma_start(t[:], src if src is not None else io[name][:])
            return t

        wq_sb = [const_tile(f"wq{i}", [128, DPC], F32R, io["wqT"][i * 128:(i + 1) * 128, :]) for i in range(8)]
        wk_sb = [const_tile(f"wk{i}", [128, DPC], F32R, io["wkT"][i * 128:(i + 1) * 128, :]) for i in range(8)]
        wv_sb = [const_tile(f"wv{i}", [128, DPC], F32R, io["wvT"][i * 128:(i + 1) * 128, :]) for i in range(8)]
        sxp = const_tile("sxp", [128, NTT])
        pswapT = const_tile("pswapT", [128, 128], F32R)
        negI = const_tile("negI", [128, 128], BF16)
        umask = const_tile("umask", [128, 4 * QB], BF16)
        sel2 = const_tile("sel2", [33, 128], F32R)
        ones1 = const_tile("ones1", [1, 128], F32R)
        ident = const_tile("ident", [128, 128], F32R)

        # ---- lifetime-scoped big buffers (opened in reverse-close order)
        es_big1 = ExitStack()
        big1 = es_big1.enter_context(tc.tile_pool(name="big1", bufs=1))
        es_va = ExitStack()
        vap = es_va.enter_context(tc.tile_pool(name="vap", bufs=1))
        es_qkv = ExitStack()
        qkvp = es_qkv.enter_context(tc.tile_pool(name="qkv", bufs=1))
        qT = qkvp.tile([128, NT], F32R, name="qT", tag="qT")
        kT = qkvp.tile([128, NT], F32R, name="kT", tag="kT")
        vT = qkvp.tile([128, NT], F32R, name="vT", tag="vT")
        xiT = [big1.tile([128, NT], BF16, name=f"xiT{i}", tag=f"xiT{i}") for i in range(8)]

        # ======== P1: x quantization (natural layout) + transpose DMAs
        xi2d = dpool.tile([NT, D], BF16, name="xi2d", tag="xi2d")
        with tc.tile_pool(name="p1", bufs=2) as p1:
            for tt in range(NTT):
                xt = p1.tile([128, D], F32, name="xt", tag="xt")
                nc.gpsimd.dma_start(xt[:], io["x"][tt * 128:(tt + 1) * 128, :])
                y = p1.tile([128, D], F32, name="y", tag="y")
                nc.gpsimd.tensor_scalar(y[:], xt[:], sxp[:, tt:tt + 1], RC,
                                        OP.mult, OP.add)
                xi = p1.tile([128, D], BF16, name="xi", tag="xi")
                nc.gpsimd.tensor_scalar(xi[:], y[:], RC, None, OP.subtract)
                nc.gpsimd.dma_start(xi2d[tt * 128:(tt + 1) * 128, :], xi[:])
                for i in range(8):
                    nc.sync.dma_start(xiT[i][:, tt * 128:(tt + 1) * 128],
                                      xi2d[tt * 128:(tt + 1) * 128,
                                           i * 128:(i + 1) * 128],
                                      transpose=True)

        # ======== P3: projections q,k,v (f32r); fold 1/s_x into copy-out
        with tc.tile_pool(name="p3", bufs=2) as p3, \
             tc.tile_pool(name="p3p", bufs=3, space="PSUM") as p3p, \
             tc.tile_pool(name="p3i", bufs=2) as p3i:
            for tb in range(NTB):
                sl = slice(tb * TB, (tb + 1) * TB)
                isxb = p3i.tile([128, TB], F32, name="isxb", tag="isxb")
                nc.sync.dma_start(isxb[:], io["isx"][:, sl])
                xf = []
                for i in range(8):
                    t = p3.tile([128, TB], F32R, name=f"xf{i}", tag=f"xf{i}")
                    nc.vector.tensor_copy(t[:], xiT[i][:, sl])
                    xf.append(t)
                for w_sb, dstT in ((wq_sb, qT), (wk_sb, kT), (wv_sb, vT)):
                    pp = p3p.tile([128, TB], F32, name="pp", tag="pp")
                    for i in range(8):
                        nc.tensor.matmul(pp[:], w_sb[i][:], xf[i][:],
                                         start=(i == 0), stop=(i == 7))
                    nc.vector.tensor_tensor(dstT[:, sl], pp[:], isxb[:], OP.mult)

        # ======== P4: RoPE on q, k
        qR = big1.tile([128, NT], F32R, name="qR", tag="xiT0")
        kR = big1.tile([128, NT], F32R, name="kR", tag="xiT1")
        with tc.tile_pool(name="p4", bufs=3) as p4, \
             tc.tile_pool(name="p4p", bufs=3, space="PSUM") as p4p, \
             tc.tile_pool(name="p4m", bufs=3) as p4m:
            for tb in range(NTB):
                sl = slice(tb * TB, (tb + 1) * TB)
                cm = p4m.tile([128, TB], F32, name="cm", tag="cm")
                nc.sync.dma_start(cm[:], io["cmap"][:, sl])
                sm = p4m.tile([128, TB], F32, name="sm", tag="sm")
                nc.sync.dma_start(sm[:], io["smap"][:, sl])
                for srcT, dstR in ((qT, qR), (kT, kR)):
                    swp = p4p.tile([128, TB], F32, name="swp", tag="swp")
                    nc.tensor.matmul(swp[:], pswapT[:], srcT[:, sl],
                                     start=True, stop=True)
                    tmp = p4.tile([128, TB], F32, name="tmp", tag="tmp")
                    nc.vector.tensor_tensor(tmp[:], srcT[:, sl], cm[:], OP.mult)
                    tmp2 = p4.tile([128, TB], F32, name="tmp2", tag="tmp2")
                    nc.vector.tensor_tensor(tmp2[:], swp[:], sm[:], OP.mult)
                    nc.gpsimd.tensor_tensor(dstR[:, sl], tmp[:], tmp2[:], OP.add)

        # ======== P5: V transpose to natural layout (bf16)
        ones_col = cpool.tile([128, 1], BF16, name="ones_col", tag="ones_col")
        nc.vector.memset(ones_col[:], 1.0)
        vaug = [[None] * NTT for _ in range(HPC)]
        with tc.tile_pool(name="p5p", bufs=3, space="PSUM") as p5p:
            for kt in range(NTT):
                vtp = p5p.tile([128, 128], F32, name="vtp", tag="vtp")
                nc.tensor.transpose(r32(vtp[:]), vT[:, kt * 128:(kt + 1) * 128],
                                    ident[:])
                for h in range(HPC):
                    va = vap.tile([128, HD], BF16, name=f"va{h}_{kt}", tag=f"va{h}_{kt}")
                    if h == 0:
                        nc.scalar.copy(va[:], vtp[:, 0:HD])
                    else:
                        nc.vector.tensor_copy(va[:], vtp[:, HD:128])
                    vaug[h][kt] = va
        es_qkv.close()

        # ======== P6: attention
        out_n = big1.tile([128, NT], F32R, name="out_n", tag="xiT2")
        with tc.tile_pool(name="p6a", bufs=3) as p6a, \
             tc.tile_pool(name="p6s", bufs=2, space="PSUM") as p6s, \
             tc.tile_pool(name="p6o", bufs=1, space="PSUM") as p6o:
            for b in range(B):
                for qb in range(NQB):
                    qsl = slice(b * T + qb * QB, b * T + (qb + 1) * QB)
                    nkt = 4 * qb + 4
                    psA0 = p6o.tile([128, QB], F32, name="psA0", tag="psA0")
                    psA1 = p6o.tile([128, QB], F32, name="psA1", tag="psA1")
                    psRS0 = p6o.tile([1, QB], F32, name="psRS0", tag="psRS0")
                    psRS1 = p6o.tile([33, QB], F32, name="psRS1", tag="psRS1")
                    for kl in range(nkt):
                        kt = b * NKT + kl
                        ksl = slice(kt * 128, (kt + 1) * 128)
                        psS = p6s.tile([128, 2 * QB], F32, name="psS", tag="psS")
                        diag = kl >= 4 * qb
                        for h in range(HPC):
                            hsl = slice(h * HD, (h + 1) * HD)
                            ssl = slice(h * QB, (h + 1) * QB)
                            nc.tensor.matmul(psS[:, ssl], kR[hsl, ksl],
                                             qR[hsl, qsl],
                                             start=True, stop=not diag)
                            if diag:
                                v = kl - 4 * qb
                                nc.tensor.matmul(
                                    psS[:, ssl], negI[:],
                                    umask[:, v * QB:(v + 1) * QB],
                                    start=False, stop=True)
                        A = p6a.tile([128, 2 * QB], BF16, name="A", tag="A")
                        nc.scalar.activation(A[:], psS[:],
                                             mybir.ActivationFunctionType.Exp,
                                             scale=1.0 / math.sqrt(HD))
                        st, sp = kl == 0, kl == nkt - 1
                        nc.tensor.matmul(psA0[0:HD, :], vaug[0][kt][:],
                                         A[:, 0:QB], start=st, stop=sp,
                                         tile_position=(0, 0))
                        nc.tensor.matmul(psA1[HD:128, :], vaug[1][kt][:],
                                         A[:, QB:2 * QB], start=st, stop=sp,
                                         tile_position=(0, 64))
                        nc.tensor.matmul(psRS0[0:1, :], ones_col[:],
                                         A[:, 0:QB], start=st, stop=sp,
                                         tile_position=(0, 0))
                        nc.tensor.matmul(psRS1[32:33, :], ones_col[:],
                                         A[:, QB:2 * QB], start=st, stop=sp,
                                         tile_position=(0, 32))
                    ou_blk = p6a.tile([128, QB], F32, name="ou_blk", tag="ou_blk")
                    nc.scalar.copy(ou_blk[0:HD, :], psA0[0:HD, :])
                    nc.scalar.copy(ou_blk[HD:128, :], psA1[HD:128, :])
                    rsi = p6a.tile([33, QB], F32R, name="rsi", tag="rsi")
                    nc.vector.tensor_copy(rsi[0:32, :], umask[0:32, 0:QB])
                    nc.vector.reciprocal(rsi[0:1, :], psRS0[0:1, :])
                    nc.vector.reciprocal(rsi[32:33, :], psRS1[32:33, :])
                    brs = p6s.tile([128, QB], F32, name="brs", tag="psS")
                    nc.tensor.matmul(brs[:], sel2[:], rsi[:],
                                     start=True, stop=True)
                    nc.vector.tensor_tensor(out_n[:, qsl], ou_blk[:], brs[:],
                                            OP.mult)
        es_va.close()

        # ======== P7: out-quant + collectives + wo
        xio = big1.tile([128, NT], BF16, name="xio", tag="xiT3")
        with tc.tile_pool(name="p7p", bufs=2, space="PSUM") as p7p, \
             tc.tile_pool(name="p7", bufs=1) as p7:
            # out-quant absmax over partition dim via PE transpose
            amax = p7.tile([128, NTT], F32, name="amax", tag="amax")
            for ot in range(NTT):
                tp = p7p.tile([128, 128], F32, name="tp", tag="tp")
                nc.tensor.transpose(r32(tp[:]), out_n[:, ot * 128:(ot + 1) * 128],
                                    ident[:])
                nc.vector.tensor_reduce(amax[:, ot:ot + 1], tp[:],
                                        mybir.AxisListType.X, OP.max,
                                        apply_absolute_value=True)
            ar_in = dpool.tile([128, NTT], F32, name="ar_in", tag="ar_in")
            ar_out = dpool.tile([128, NTT], F32, name="ar_out", tag="ar_out", addr_space="Shared")
            nc.sync.dma_start(ar_in[:], amax[:])
            nc.gpsimd.collective_compute(
                "AllReduce", OP.max, replica_groups=RG,
                ins=[ar_in[:].opt()], outs=[ar_out[:].opt()])
            gmax = p7.tile([128, NTT], F32, name="gmax", tag="gmax")
            nc.sync.dma_start(gmax[:], ar_out[:])
            iso_p = p7.tile([128, NTT], F32, name="iso_p", tag="iso_p")
            nc.vector.tensor_scalar(iso_p[:], gmax[:], 1e-5, 1.0 / 127.0,
                                    OP.add, OP.mult)
            so_p = p7.tile([128, NTT], F32R, name="so_p", tag="so_p")
            nc.vector.reciprocal(so_p[:], iso_p[:])
            sop_t = p7p.tile([NTT, 128], F32, name="tp", tag="tp")
            nc.tensor.transpose(r32(sop_t[:]), so_p[:], ident[:])
            so_sq = p7.tile([NTT, 128], F32R, name="so_sq", tag="so_sq")
            nc.scalar.copy(so_sq[:], sop_t[:])
            so_row = p7.tile([1, NT], F32R, name="so_row", tag="so_row")
            for j in range(NTT):
                nc.sync.dma_start(so_row[0:1, j * 128:(j + 1) * 128],
                                  so_sq[j:j + 1, :])
            iso_row = p7.tile([1, NT], F32R, name="iso_row", tag="iso_row")
            nc.vector.reciprocal(iso_row[:], so_row[:])

            # quantize out_n -> xio (integer-valued bf16)
            for tb in range(NTB):
                sl = slice(tb * TB, (tb + 1) * TB)
                bso = p7p.tile([128, TB], F32, name="brs", tag="brs")
                nc.tensor.matmul(bso[:], ones1[:], so_row[:, sl],
                                 start=True, stop=True)
                yq = p7.tile([128, TB], F32, name=f"yq{tb % 2}", tag=f"yq{tb % 2}")
                nc.vector.tensor_tensor(yq[:], out_n[:, sl], bso[:], OP.mult)
                nc.gpsimd.tensor_scalar(xio[:, sl], yq[:], RC, RC,
                                        OP.add, OP.subtract)

            # ======== P9: AllGather
            ag_in = dpool.tile([128, NT], BF16, name="ag_in", tag="ag_in")
            ag_out = dpool.tile([N_CORES * 128, NT], BF16, name="ag_out", tag="ag_out", addr_space="Shared")
            nc.sync.dma_start(ag_in[:], xio[:])
            nc.gpsimd.collective_compute(
                "AllGather", OP.bypass, replica_groups=RG,
                ins=[ag_in[:].opt()], outs=[ag_out[:].opt()])

            # ======== P10: wo projection (bf16) + final scale
            with tc.tile_pool(name="pA", bufs=2) as pA, \
                 tc.tile_pool(name="pAg", bufs=1) as pAg, \
                 tc.tile_pool(name="pAp", bufs=2, space="PSUM") as pAp:
                wo_sb = [pAg.tile([128, DPC], BF16, name=f"wo{i}", tag=f"wo{i}")
                         for i in range(8)]
                g_sb = [big1.tile([128, NT], BF16, name=f"g{i}", tag=f"xiT{i}") for i in range(8)]
                for i in range(8):
                    nc.sync.dma_start(wo_sb[i][:],
                                      io["woT"][i * 128:(i + 1) * 128, :])
                    nc.sync.dma_start(g_sb[i][:],
                                      ag_out[i * 128:(i + 1) * 128, :])
                iso_bc = pAg.tile([128, NT], F32, name="iso_bc", tag="iso_bc")
                for tb in range(NTB):
                    sl = slice(tb * TB, (tb + 1) * TB)
                    bi = pAp.tile([128, TB], F32, name="bi", tag="bi")
                    nc.tensor.matmul(bi[:], ones1[:], iso_row[:, sl],
                                     start=True, stop=True)
                    nc.scalar.copy(iso_bc[:, sl], bi[:])
                for tb in range(NTB):
                    sl = slice(tb * TB, (tb + 1) * TB)
                    pw = pAp.tile([128, TB], F32, name="pw", tag="pw")
                    for i in range(8):
                        nc.tensor.matmul(pw[:], wo_sb[i][:], g_sb[i][:, sl],
                                         start=(i == 0), stop=(i == 7))
                    fin = pA.tile([128, TB], F32, name="fin", tag="fin")
                    nc.vector.tensor_tensor(fin[:], pw[:], iso_bc[:, sl],
                                            OP.mult)
                    nc.sync.dma_start(out[:, sl], fin[:])
        es_big1.close()

    return nc


_CACHE = {}


def kernel(x, cos, sin, wq_w, wk_w, wv_w, wo_w):
    x = np.asarray(x, np.float32)
    cos = np.asarray(cos, np.float32)   # [T, 32]
    sin = np.asarray(sin, np.float32)
    xf = np.ascontiguousarray(x.reshape(NT, D))

    amax = np.abs(xf).max(-1) + 1e-5
    sx = (127.0 / amax).astype(np.float32)
    isx = (amax / 127.0).astype(np.float32)
    sxp = np.ascontiguousarray(sx.reshape(NTT, 128).T)
    isx_bc = np.ascontiguousarray(np.broadcast_to(isx[None, :], (128, NT)))

    # RoPE maps from the provided cos/sin tables
    cm64 = np.repeat(cos.T, 2, axis=0)            # [64, T]
    sm64 = np.repeat(sin.T, 2, axis=0)
    # rows: [64 dims for head-even][64 dims for head-odd]; cols: [b0 | b1]
    cmap = np.tile(np.concatenate([cm64, cm64], axis=0), (1, B)).astype(np.float32)
    smap = np.tile(np.concatenate([sm64, sm64], axis=0), (1, B)).astype(np.float32)

    P = np.zeros((128, 128), np.float32)
    for j in range(64):
        P[2 * j, 2 * j + 1] = -1.0
        P[2 * j + 1, 2 * j] = 1.0
    pswapT = np.ascontiguousarray(P.T)
    negI = (NEG * np.eye(128)).astype(ml_dtypes.bfloat16)
    kk = np.arange(128)[:, None]
    qq = np.arange(QB)[None, :]
    um = np.concatenate([((v * 128 + kk) > qq).astype(np.float32)
                         for v in range(4)], axis=1).astype(ml_dtypes.bfloat16)
    sel2 = np.zeros((33, 128), np.float32)
    sel2[0, 0:HD] = 1.0
    sel2[32, HD:128] = 1.0
    ones1 = np.ones((1, 128), np.float32)
    ident = np.eye(128, dtype=np.float32)

    wq_e, wk_e, wv_e, wo_e = (_quant_w(np.asarray(w, np.float32))
                              for w in (wq_w, wk_w, wv_w, wo_w))

    if "nc" not in _CACHE:
        nc0 = build_nc()
        nc0.finalize()
        _CACHE["nc"] = nc0
    nc = _CACHE["nc"]

    in_maps = []
    for c in range(N_CORES):
        hs = slice(c * DPC, (c + 1) * DPC)
        in_maps.append({
            "x": xf, "sxp": sxp, "isx": isx_bc,
            "wqT": np.ascontiguousarray(wq_e[hs, :].T),
            "wkT": np.ascontiguousarray(wk_e[hs, :].T),
            "wvT": np.ascontiguousarray(wv_e[hs, :].T),
            "woT": np.ascontiguousarray(wo_e[hs, :].T).astype(ml_dtypes.bfloat16),
            "cmap": cmap, "smap": smap, "pswapT": pswapT, "negI": negI,
            "umask": um, "sel2": sel2, "ones1": ones1, "ident": ident,
        })

    res = run_bass_kernel_spmd(nc, in_maps, core_ids=list(range(N_CORES)))
    outp = np.empty((NT, D), np.float32)
    for c in range(N_CORES):
        outp[:, c * DPC:(c + 1) * DPC] = res.results[c]["out"].T
    return outp.reshape(B, T, D)



# revision 19
# speedup vs baseline: 2.4906x; 2.4906x over previous
"""BitNet-style attention layer (B=2, T=2048, D=1024, 16 heads, RoPE, causal)
on 8 TRN2 NeuronCores.

Sharding: head-parallel attention (2 heads/core); wo is computed per-core for
an o-slice after an AllGather of the int8-valued (bf16-stored) quantized
attention output.  Per-token output-quant scales come from a chunked
AllReduce(max) pipelined behind the attention of later chunks.

Pipeline layout (single fused graph):
  A: per 512-token block: DMA x, quantize (scalar/vector), PE-transpose to
     f32r, QKV projections, in-place RoPE, V-transpose (+ones column for
     fused softmax row-sums).
  B: attention per (batch, q-block) with the epilogue (amax -> AllReduce ->
     quantize -> AllGather -> wo) chunked 4x and interleaved.
"""

import math
from contextlib import ExitStack

import ml_dtypes
import numpy as np

import concourse.bass as bass
import concourse.bacc as bacc_mod
import concourse.mybir as mybir
import concourse.tile as tile
from concourse.bass_utils import run_bass_kernel_spmd

F32 = mybir.dt.float32
F32R = mybir.dt.float32r
BF16 = mybir.dt.bfloat16
OP = mybir.AluOpType
ACT = mybir.ActivationFunctionType

B, T, D = 2, 2048, 1024
NT = B * T              # 4096 tokens
NH, HD = 16, 64
HDP1 = HD + 1           # V augmented with a ones column (fused row-sum)
N_CORES = 8
HPC = NH // N_CORES     # heads per core = 2
DPC = HPC * HD          # dims per core = 128
RC = 12582912.0         # 1.5*2^23: round-to-nearest-even constant
NEG = -1e30

TB = 512                # token block (matmul N)
NTB = NT // TB          # 8
NTT = NT // 128         # 32 token tiles
QB = 512                # q block
NQB = T // QB           # 4 per batch
NKT = T // 128          # 16 k tiles per batch

NC_CHUNKS = 4           # epilogue pipeline chunks
TPC = NT // NC_CHUNKS   # tokens per chunk = 1024
TTPC = TPC // 128       # token tiles per chunk = 8
TBPC = TPC // TB        # token blocks per chunk = 2


def _quant_w(w):
    O, I = w.shape
    wg = w.reshape(O, I // 128, 128)
    ws = np.abs(wg).mean(-1, keepdims=True) + 1e-5
    wq = np.clip(np.round(wg / ws), -1.0, 1.0) * ws
    return wq.reshape(O, I).astype(np.float32)


def build_nc():
    nc = bacc_mod.Bacc(num_devices=N_CORES)
    io = {}

    def inp(name, shape, dt=F32):
        io[name] = nc.dram_tensor(name, shape, dt, kind="ExternalInput")

    inp("x", [NT, D])
    inp("sxp", [128, NTT])
    inp("isx", [128, NT])
    inp("wqT", [D, DPC], F32R)
    inp("wkT", [D, DPC], F32R)
    inp("wvT", [D, DPC], F32R)
    inp("woT", [D, DPC], BF16)
    inp("cmap", [128, NT])
    inp("smap", [128, NT])
    inp("pswapT", [128, 128], F32R)
    inp("negI", [128, 128], BF16)
    inp("umask", [128, 4 * QB], BF16)
    inp("sel2", [33, 128], F32R)
    inp("ones1", [1, 128], F32R)
    inp("identR", [128, 128], F32R)
    inp("identB", [128, 128], BF16)
    out = nc.dram_tensor("out", [DPC, NT], F32, kind="ExternalOutput")

    r32 = lambda ap: ap.bitcast(F32R)
    RG = [list(range(N_CORES))]

    with nc.allow_low_precision(reason="f32r matmul pipeline (FP22 mantissa is ample here)"), \
         tile.TileContext(nc) as tc, ExitStack() as top:
        cpool = top.enter_context(tc.tile_pool(name="const", bufs=1))
        dpool = top.enter_context(tc.tile_pool(name="dram", bufs=1, space="DRAM"))

        # ---- constants
        def const_tile(name, shape, dt=F32, src=None):
            t = cpool.tile(shape, dt, tag=name)
            nc.gpsimd.dma_start(t[:], src if src is not None else io[name][:])
            return t

        wq_sb = [const_tile(f"wq{i}", [128, DPC], F32R, io["wqT"][i * 128:(i + 1) * 128, :]) for i in range(8)]
        wk_sb = [const_tile(f"wk{i}", [128, DPC], F32R, io["wkT"][i * 128:(i + 1) * 128, :]) for i in range(8)]
        wv_sb = [const_tile(f"wv{i}", [128, DPC], F32R, io["wvT"][i * 128:(i + 1) * 128, :]) for i in range(8)]
        wo_sb = [const_tile(f"wo{i}", [128, DPC], BF16, io["woT"][i * 128:(i + 1) * 128, :]) for i in range(8)]
        sxp = const_tile("sxp", [128, NTT])
        pswapT = const_tile("pswapT", [128, 128], F32R)
        negI = const_tile("negI", [128, 128], BF16)
        umask = const_tile("umask", [128, 4 * QB], BF16)
        sel2 = const_tile("sel2", [33, 128], F32R)
        ones1 = const_tile("ones1", [1, 128], F32R)
        identR = const_tile("identR", [128, 128], F32R)
        identB = const_tile("identB", [128, 128], BF16)

        # ---- persistent SBUF state
        es_qk = ExitStack()
        qkp = es_qk.enter_context(tc.tile_pool(name="qk", bufs=1))
        qT = qkp.tile([128, NT], F32R, name="qT", tag="qT")
        kT = qkp.tile([128, NT], F32R, name="kT", tag="kT")
        es_va = ExitStack()
        vap = es_va.enter_context(tc.tile_pool(name="vap", bufs=1))
        vaug = [[None] * NTT for _ in range(HPC)]
        es_v = ExitStack()
        vp = es_v.enter_context(tc.tile_pool(name="vp", bufs=1))
        vT = vp.tile([128, NT], F32R, name="vT", tag="vT")

        # ---- per-chunk DRAM collective buffers
        ar_in = [dpool.tile([128, TTPC], F32, name=f"ar_in{c}", tag=f"ar_in{c}")
                 for c in range(NC_CHUNKS)]
        ar_out = [dpool.tile([128, TTPC], F32, name=f"ar_out{c}", tag=f"ar_out{c}",
                             addr_space="Shared") for c in range(NC_CHUNKS)]
        ag_in = [dpool.tile([128, TPC], BF16, name=f"ag_in{c}", tag=f"ag_in{c}")
                 for c in range(NC_CHUNKS)]
        ag_out = [dpool.tile([N_CORES * 128, TPC], BF16, name=f"ag_out{c}",
                             tag=f"ag_out{c}", addr_space="Shared")
                  for c in range(NC_CHUNKS)]

        # ======== Phase A: quantize x, transpose, QKV proj, RoPE, V-transpose
        with tc.tile_pool(name="pxt", bufs=2) as pxt, \
             tc.tile_pool(name="pxf", bufs=2) as pxf, \
             tc.tile_pool(name="pm", bufs=2) as pm, \
             tc.tile_pool(name="ptmp", bufs=2) as ptmp, \
             tc.tile_pool(name="ptp", bufs=2, space="PSUM") as ptp, \
             tc.tile_pool(name="ppp", bufs=2, space="PSUM") as ppp, \
             tc.tile_pool(name="prp", bufs=2, space="PSUM") as prp, \
             tc.tile_pool(name="pvt", bufs=2, space="PSUM") as pvt:
            for tb in range(NTB):
                sl = slice(tb * TB, (tb + 1) * TB)
                xf = [pxf.tile([128, TB], F32R, name=f"xf{i}", tag=f"xf{i}")
                      for i in range(8)]
                for lt in range(4):
                    tt = tb * 4 + lt
                    xt = pxt.tile([128, D], F32, name="xt", tag="xt")
                    nc.sync.dma_start(xt[:], io["x"][tt * 128:(tt + 1) * 128, :])
                    y = pxt.tile([128, D], F32, name="y", tag="y")
                    nc.scalar.activation(y[:], xt[:], ACT.Copy, bias=RC,
                                         scale=sxp[:, tt:tt + 1])
                    xi = pxt.tile([128, D], BF16, name="xi", tag="xi")
                    if tt % 2 == 0:
                        nc.scalar.activation(xi[:], y[:], ACT.Copy, bias=-RC)
                    else:
                        nc.vector.tensor_scalar(xi[:], y[:], RC, None,
                                                OP.subtract)
                    for i in range(8):
                        tp = ptp.tile([128, 128], BF16, name="tp", tag="tp")
                        nc.tensor.transpose(tp[:], xi[:, i * 128:(i + 1) * 128],
                                            identB[:])
                        dst = xf[i][:, lt * 128:(lt + 1) * 128]
                        if i < 3:
                            nc.scalar.copy(dst, tp[:])
                        else:
                            nc.vector.tensor_copy(dst, tp[:])
                # projections
                isxb = pm.tile([128, TB], F32, name="isxb", tag="isxb")
                nc.sync.dma_start(isxb[:], io["isx"][:, sl])
                cm = pm.tile([128, TB], F32, name="cm", tag="cm")
                nc.sync.dma_start(cm[:], io["cmap"][:, sl])
                sm = pm.tile([128, TB], F32, name="sm", tag="sm")
                nc.sync.dma_start(sm[:], io["smap"][:, sl])
                for w_sb, dstT in ((wq_sb, qT), (wk_sb, kT), (wv_sb, vT)):
                    pp = ppp.tile([128, TB], F32, name="pp", tag="pp")
                    for i in range(8):
                        nc.tensor.matmul(pp[:], w_sb[i][:], xf[i][:],
                                         start=(i == 0), stop=(i == 7))
                    nc.vector.tensor_tensor(dstT[:, sl], pp[:], isxb[:],
                                            OP.mult)
                # RoPE in place on q, k
                for srcT in (qT, kT):
                    swp = prp.tile([128, TB], F32, name="swp", tag="swp")
                    nc.tensor.matmul(swp[:], pswapT[:], srcT[:, sl],
                                     start=True, stop=True)
                    tmp = ptmp.tile([128, TB], F32, name="tmp", tag="tmp")
                    nc.vector.tensor_tensor(tmp[:], srcT[:, sl], cm[:], OP.mult)
                    tmp2 = ptmp.tile([128, TB], F32, name="tmp2", tag="tmp2")
                    nc.vector.tensor_tensor(tmp2[:], swp[:], sm[:], OP.mult)
                    nc.gpsimd.tensor_tensor(srcT[:, sl], tmp[:], tmp2[:], OP.add)
                # V transpose (+ones column) for this block's 4 k-tiles
                for lt in range(4):
                    kt = tb * 4 + lt
                    vtp = pvt.tile([128, 128], F32, name="vtp", tag="vtp")
                    nc.tensor.transpose(r32(vtp[:]),
                                        vT[:, kt * 128:(kt + 1) * 128],
                                        identR[:])
                    va0 = vap.tile([128, HDP1], BF16, name=f"va0_{kt}",
                                   tag=f"va0_{kt}")
                    nc.vector.memset(va0[:, HD:HDP1], 1.0)
                    nc.scalar.copy(va0[:, 0:HD], vtp[:, 0:HD])
                    va1 = vap.tile([128, HDP1], BF16, name=f"va1_{kt}",
                                   tag=f"va1_{kt}")
                    nc.vector.memset(va1[:, HD:HDP1], 1.0)
                    nc.vector.tensor_copy(va1[:, 0:HD], vtp[:, HD:128])
                    vaug[0][kt] = va0
                    vaug[1][kt] = va1
        es_v.close()

        # ======== Phase B: attention with chunked, pipelined epilogue
        es_b = ExitStack()
        big = es_b.enter_context(tc.tile_pool(name="big", bufs=1))
        out_n = big.tile([128, NT], F32R, name="out_n", tag="out_n")
        xio = big.tile([128, NT], BF16, name="xio", tag="xio")
        rsi = big.tile([33, QB], F32, name="rsi", tag="rsi")
        rsr = big.tile([33, QB], F32R, name="rsr", tag="rsr")
        nc.vector.memset(rsi[:], 1.0)
        prow = es_b.enter_context(tc.tile_pool(name="prow", bufs=2))
        so_rows = [None] * NC_CHUNKS
        iso_rows = [None] * NC_CHUNKS
        psSp = es_b.enter_context(tc.tile_pool(name="psS", bufs=2, space="PSUM"))
        pAcc = es_b.enter_context(tc.tile_pool(name="pAcc", bufs=1, space="PSUM"))
        ppost = es_b.enter_context(tc.tile_pool(name="ppost", bufs=2, space="PSUM"))
        pA = es_b.enter_context(tc.tile_pool(name="pA", bufs=3))
        pscl = es_b.enter_context(tc.tile_pool(name="pscl", bufs=2))
        pyq = es_b.enter_context(tc.tile_pool(name="pyq", bufs=2))
        pg = es_b.enter_context(tc.tile_pool(name="pg", bufs=2))
        pfin = es_b.enter_context(tc.tile_pool(name="pfin", bufs=2))

        def attn(b, qb):
            qsl = slice(b * T + qb * QB, b * T + (qb + 1) * QB)
            nkt = 4 * qb + 4
            psA0 = pAcc.tile([128, QB], F32, name="psA0", tag="psA0")
            psA1 = pAcc.tile([128, QB], F32, name="psA1", tag="psA1")
            for kl in range(nkt):
                kt = b * NKT + kl
                ksl = slice(kt * 128, (kt + 1) * 128)
                psS = psSp.tile([128, 2 * QB], F32, name="psS", tag="psS")
                diag = kl >= 4 * qb
                for h in range(HPC):
                    hsl = slice(h * HD, (h + 1) * HD)
                    ssl = slice(h * QB, (h + 1) * QB)
                    nc.tensor.matmul(psS[:, ssl], kT[hsl, ksl], qT[hsl, qsl],
                                     start=True, stop=not diag)
                    if diag:
                        v = kl - 4 * qb
                        nc.tensor.matmul(psS[:, ssl], negI[:],
                                         umask[:, v * QB:(v + 1) * QB],
                                         start=False, stop=True)
                A = pA.tile([128, 2 * QB], BF16, name="A", tag="A")
                nc.scalar.activation(A[:], psS[:], ACT.Exp,
                                     scale=1.0 / math.sqrt(HD))
                st, sp = kl == 0, kl == nkt - 1
                nc.tensor.matmul(psA0[0:HDP1, :], vaug[0][kt][:],
                                 A[:, 0:QB], start=st, stop=sp)
                nc.tensor.matmul(psA1[0:HDP1, :], vaug[1][kt][:],
                                 A[:, QB:2 * QB], start=st, stop=sp)
            # epilogue: fused row-sums live at psA0[HD], psA1[HD]
            nc.scalar.copy(rsi[0:1, :], psA0[HD:HDP1, :])
            nc.scalar.copy(rsi[32:33, :], psA1[HD:HDP1, :])
            nc.vector.reciprocal(rsr[:], rsi[:])
            brs = ppost.tile([128, QB], F32, name="post", tag="post")
            nc.tensor.matmul(brs[:], sel2[:], rsr[:], start=True, stop=True)
            brs_sb = pyq.tile([128, QB], F32, name="brs_sb", tag="brs_sb")
            nc.scalar.copy(brs_sb[:], brs[:])
            nc.vector.tensor_tensor(out_n[0:HD, qsl], psA0[0:HD, :],
                                    brs_sb[0:HD, :], OP.mult)
            nc.vector.tensor_tensor(out_n[HD:128, qsl], psA1[0:HD, :],
                                    brs_sb[HD:128, :], OP.mult)

        def amax_ar(c):
            am = pscl.tile([128, TTPC], F32, name="am", tag="am")
            for j in range(TTPC):
                gtt = c * TTPC + j
                tp = ppost.tile([128, QB], F32, name="post", tag="post")
                nc.tensor.transpose(r32(tp[:, 0:128]),
                                    out_n[:, gtt * 128:(gtt + 1) * 128],
                                    identR[:])
                nc.vector.tensor_reduce(am[:, j:j + 1], tp[:, 0:128],
                                        mybir.AxisListType.X, OP.max,
                                        apply_absolute_value=True)
            nc.sync.dma_start(ar_in[c][:], am[:])
            nc.gpsimd.collective_compute(
                "AllReduce", OP.max, replica_groups=RG,
                ins=[ar_in[c][:].opt()], outs=[ar_out[c][:].opt()])

        def squag(c):
            csl = slice(c * TPC, (c + 1) * TPC)
            gmax = pscl.tile([128, TTPC], F32, name="gmax", tag="gmax")
            nc.sync.dma_start(gmax[:], ar_out[c][:])
            iso_p = pscl.tile([128, TTPC], F32R, name="iso_p", tag="iso_p")
            nc.vector.tensor_scalar(iso_p[:], gmax[:], 1e-5, 1.0 / 127.0,
                                    OP.add, OP.mult)
            so_p = pscl.tile([128, TTPC], F32R, name="so_p", tag="so_p")
            nc.vector.reciprocal(so_p[:], iso_p[:])
            tso = ppost.tile([128, QB], F32, name="post", tag="post")
            nc.tensor.transpose(r32(tso[0:TTPC, 0:128]), so_p[:], identR[:])
            so_sq = pscl.tile([TTPC, 128], F32R, name="so_sq", tag="so_sq")
            nc.scalar.copy(so_sq[:], tso[0:TTPC, 0:128])
            tiso = ppost.tile([128, QB], F32, name="post", tag="post")
            nc.tensor.transpose(r32(tiso[0:TTPC, 0:128]), iso_p[:],
                                identR[:])
            iso_sq = pscl.tile([TTPC, 128], F32R, name="iso_sq", tag="iso_sq")
            nc.scalar.copy(iso_sq[:], tiso[0:TTPC, 0:128])
            so_rows[c] = prow.tile([1, TPC], F32R, name="so_row", tag="so_row")
            iso_rows[c] = prow.tile([1, TPC], F32R, name="iso_row", tag="iso_row")
            for j in range(TTPC):
                col = j * 128
                nc.sync.dma_start(so_rows[c][0:1, col:col + 128],
                                  so_sq[j:j + 1, :])
                nc.sync.dma_start(iso_rows[c][0:1, col:col + 128],
                                  iso_sq[j:j + 1, :])
            for tl in range(TBPC):
                tbg = c * TBPC + tl
                sl = slice(tbg * TB, (tbg + 1) * TB)
                bso = ppost.tile([128, QB], F32, name="post", tag="post")
                nc.tensor.matmul(bso[:], ones1[:],
                                 so_rows[c][:, tl * TB:(tl + 1) * TB],
                                 start=True, stop=True)
                yq = pyq.tile([128, TB], F32, name="yq", tag="yq")
                nc.vector.tensor_tensor(yq[:], out_n[:, sl], bso[:], OP.mult)
                nc.vector.tensor_scalar(xio[:, sl], yq[:], RC, RC,
                                        OP.add, OP.subtract)
            nc.sync.dma_start(ag_in[c][:], xio[:, csl])
            nc.gpsimd.collective_compute(
                "AllGather", OP.bypass, replica_groups=RG,
                ins=[ag_in[c][:].opt()], outs=[ag_out[c][:].opt()])

        def wo(c):
            for tl in range(TBPC):
                tbg = c * TBPC + tl
                sl = slice(tbg * TB, (tbg + 1) * TB)
                pw = ppost.tile([128, QB], F32, name="post", tag="post")
                for i in range(8):
                    g = pg.tile([128, TB], BF16, name=f"g{i}", tag=f"g{i}")
                    nc.sync.dma_start(g[:], ag_out[c][i * 128:(i + 1) * 128,
                                                      tl * TB:(tl + 1) * TB])
                    nc.tensor.matmul(pw[:], wo_sb[i][:], g[:],
                                     start=(i == 0), stop=(i == 7))
                bi = ppost.tile([128, QB], F32, name="post", tag="post")
                nc.tensor.matmul(bi[:], ones1[:],
                                 iso_rows[c][:, tl * TB:(tl + 1) * TB],
                                 start=True, stop=True)
                bi_sb = pyq.tile([128, TB], F32, name="bi_sb", tag="bi_sb")
                nc.scalar.copy(bi_sb[:], bi[:])
                fin = pfin.tile([128, TB], F32, name="fin", tag="fin")
                nc.vector.tensor_tensor(fin[:], pw[:], bi_sb[:], OP.mult)
                nc.sync.dma_start(out[:, sl], fin[:])

        attn(0, 0)
        attn(0, 1); amax_ar(0)
        attn(0, 2)
        attn(0, 3); amax_ar(1); squag(0)
        attn(1, 0)
        attn(1, 1); amax_ar(2); squag(1)
        attn(1, 2); wo(0)
        attn(1, 3); amax_ar(3); squag(2); wo(1)
        wo(2); squag(3); wo(3)

        es_b.close()
        es_va.close()
        es_qk.close()

    return nc


_CACHE = {}


def kernel(x, cos, sin, wq_w, wk_w, wv_w, wo_w):
    x = np.asarray(x, np.float32)
    cos = np.asarray(cos, np.float32)   # [T, 32]
    sin = np.asarray(sin, np.float32)
    xf = np.ascontiguousarray(x.reshape(NT, D))

    amax = np.abs(xf).max(-1) + 1e-5
    sx = (127.0 / amax).astype(np.float32)
    isx = (amax / 127.0).astype(np.float32)
    sxp = np.ascontiguousarray(sx.reshape(NTT, 128).T)
    isx_bc = np.ascontiguousarray(np.broadcast_to(isx[None, :], (128, NT)))

    # RoPE maps from the provided cos/sin tables
    cm64 = np.repeat(cos.T, 2, axis=0)            # [64, T]
    sm64 = np.repeat(sin.T, 2, axis=0)
    # rows: [64 dims for head-even][64 dims for head-odd]; cols: [b0 | b1]
    cmap = np.tile(np.concatenate([cm64, cm64], axis=0), (1, B)).astype(np.float32)
    smap = np.tile(np.concatenate([sm64, sm64], axis=0), (1, B)).astype(np.float32)

    P = np.zeros((128, 128), np.float32)
    for j in range(64):
        P[2 * j, 2 * j + 1] = -1.0
        P[2 * j + 1, 2 * j] = 1.0
    pswapT = np.ascontiguousarray(P.T)
    negI = (NEG * np.eye(128)).astype(ml_dtypes.bfloat16)
    kk = np.arange(128)[:, None]
    qq = np.arange(QB)[None, :]
    um = np.concatenate([((v * 128 + kk) > qq).astype(np.float32)
                         for v in range(4)], axis=1).astype(ml_dtypes.bfloat16)
    sel2 = np.zeros((33, 128), np.float32)
    sel2[0, 0:HD] = 1.0
    sel2[32, HD:128] = 1.0
    ones1 = np.ones((1, 128), np.float32)
    ident = np.eye(128, dtype=np.float32)
    identB = np.eye(128, dtype=np.float32).astype(ml_dtypes.bfloat16)

    wq_e, wk_e, wv_e, wo_e = (_quant_w(np.asarray(w, np.float32))
                              for w in (wq_w, wk_w, wv_w, wo_w))

    if "nc" not in _CACHE:
        nc0 = build_nc()
        nc0.finalize()
        _CACHE["nc"] = nc0
    nc = _CACHE["nc"]

    in_maps = []
    for c in range(N_CORES):
        hs = slice(c * DPC, (c + 1) * DPC)
        in_maps.append({
            "x": xf, "sxp": sxp, "isx": isx_bc,
            "wqT": np.ascontiguousarray(wq_e[hs, :].T),
            "wkT": np.ascontiguousarray(wk_e[hs, :].T),
            "wvT": np.ascontiguousarray(wv_e[hs, :].T),
            "woT": np.ascontiguousarray(wo_e[hs, :].T).astype(ml_dtypes.bfloat16),
            "cmap": cmap, "smap": smap, "pswapT": pswapT, "negI": negI,
            "umask": um, "sel2": sel2, "ones1": ones1, "identR": ident,
            "identB": identB,
        })

    res = run_bass_kernel_spmd(nc, in_maps, core_ids=list(range(N_CORES)))
    outp = np.empty((NT, D), np.float32)
    for c in range(N_CORES):
        outp[:, c * DPC:(c + 1) * DPC] = res.results[c]["out"].T
    return outp.reshape(B, T, D)
